# revision 1
# baseline (speedup 1.0000x reference)
"""Trainium2 Bass kernel for causal attention (scores = K @ Q^T variant).

Problem (hardcoded):
  x  [8, 2048, 2048] f32, Wk/Wq/Wv [2048, 256] f32
  per batch b: K = x_b @ Wk, Q = x_b @ Wq, V = x_b @ Wv
  w = K @ Q^T / sqrt(256), causal-masked (strict upper = -inf),
  attn = softmax(w, axis=-1), out_b = attn @ V    -> [8, 2048, 256] f32

Sharding: data-parallel over batch, one batch element per NeuronCore (8 cores).

v5 design (lessons: DMA-XBAR transpose floods the ring with 256B
descriptors ~12us/tile -> dead; SWDGE casting ~100GB/s -> only for the
last 3 x tiles; SWDGE plain f32 ~220GB/s -> weights; many small DMAs
cause semaphore-recycling serialization -> few big DMAs):
  - x tiles 0-12 arrive f32 via sync-queue HWDGE (8KB descriptors, fast),
    cast to bf16 on ACT in halves; tiles 13-15 via SWDGE bf16 cast-DMA
    behind the weights.  PE transposes the bf16 tiles (1 cycle/row).
  - Weights arrive f32 via SWDGE as 4 big DMAs, cast to bf16 on DVE in
    ec-chunks so projections can start on the first chunk.
  - Projections in bf16 (1 cycle/row), 256-col blocks, arrival-ordered;
    K^T/Q^T/V stored f32r.
  - Scores pre-transposed: sc^T[s,t] = Q^T(stationary) x K^T(moving);
    exp(sc^T) feeds attn@V directly -- no P transposes.  Ones-column in
    V yields row-sums inside the AV accumulation; reciprocal on DVE,
    scale on ACT; outputs batched 4 tiles per DMA.
  - Stage 2 is one flat stream of 40 (window, s-tile) steps with score
    matmuls emitted 3 steps ahead to hide the exp chain latency.
"""
import sys

for _p in ("/opt/trn_rl_repo",):
    if _p not in sys.path:
        sys.path.insert(0, _p)

import numpy as np

import concourse.bass as bass  # noqa: F401  (registers AP machinery)
import concourse.mybir as mybir
from concourse import bacc
from concourse.tile import TileContext
from concourse.bass_utils import run_bass_kernel_spmd
from concourse.masks import make_identity

F32 = mybir.dt.float32
F32R = mybir.dt.float32r
BF16 = mybir.dt.bfloat16

P = 128          # partitions
T = 2048         # sequence length (== E by construction of the module)
E = 2048         # embedding dim
D = 256          # head dim
EC = E // P      # 16 e-chunks
NT = T // P      # 16 t tiles
QB = 512         # stage-2 query window width
SCALE = 1.0 / 16.0   # 1/sqrt(D)
MASKVAL = -1e9

N_CORES = 8
N_HW = 13        # x tiles via HWDGE+ACT cast; the last 3 via SWDGE


def _build():
    nc = bacc.Bacc("TRN2", target_bir_lowering=False, debug=False,
                   num_devices=N_CORES)
    x_h = nc.dram_tensor("x", [T, E], F32, kind="ExternalInput")
    wk_h = nc.dram_tensor("Wk", [E, D], F32, kind="ExternalInput")
    wq_h = nc.dram_tensor("Wq", [E, D], F32, kind="ExternalInput")
    wv_h = nc.dram_tensor("Wv", [E, D], F32, kind="ExternalInput")
    y_h = nc.dram_tensor("out", [T, D], F32, kind="ExternalOutput")
    x_ap, y_ap = x_h.ap(), y_h.ap()

    with TileContext(nc) as tc:
        with tc.tile_pool(name="persist", bufs=1) as persist:
            # --- persistent tensors -------------------------------------
            wk_b = persist.tile([P, EC, D], BF16, name="wk_b")
            wq_b = persist.tile([P, EC, D], BF16, name="wq_b")
            wv_b = persist.tile([P, EC, D], BF16, name="wv_b")
            xtb = persist.tile([P, EC, T], BF16, name="xtb")    # x^T [e, t]
            kt = persist.tile([P, 2, T], F32R, name="kt")       # K^T [d, t]
            qt = persist.tile([P, 2, T], F32R, name="qt")       # Q^T [d, s]
            v_sb = persist.tile([P, NT, D + 2], F32R,
                                name="v_sb")                    # V|1|0 [s,d]
            # master mask [P, 1024]: for diag s-tile j (block cols [c0,512)),
            # slice [c0+512-128j : 1024-128j]; masked iff t_loc < 128j + p.
            maskm = persist.tile([P, 1024], F32, name="maskm")
            ident_b = persist.tile([P, P], BF16, name="ident_b")

            xb = {}

            with tc.tile_pool(name="stgp", bufs=1) as stgp, \
                 tc.tile_pool(name="s1", bufs=1) as s1, \
                 tc.tile_pool(name="s1ps", bufs=1, space="PSUM") as s1ps:
                # --- PE warmup: keep the clock ramped before x0 lands ----
                warm_src = s1.tile([P, 512], BF16, name="warm_src")
                nc.vector.memset(warm_src[:], 0.0)
                warm_ps = s1ps.tile([P, 512], F32, name="warm_ps")
                for _ in range(24):
                    nc.tensor.matmul(warm_ps[:], warm_src[:, 0:P],
                                     warm_src[:], start=True, stop=True)
                ident_f = s1.tile([P, P], F32, name="ident_f")
                make_identity(nc, ident_f[:])
                nc.vector.tensor_copy(ident_b[:], ident_f[:])

                # --- SWDGE: weights f32 (6 half DMAs), then x13-15 bf16 --
                wf = {}
                for wname, wh in (("v", wv_h), ("k", wk_h), ("q", wq_h)):
                    rap = wh.ap().rearrange("(ec p) d -> p ec d", p=P)
                    for h in range(2):
                        t = stgp.tile([P, 8, D], F32, name="w_f", tag="wf",
                                      bufs=3)
                        if wname == "v":   # quarters: earliest V0 start
                            for q in range(2):
                                nc.gpsimd.dma_start(
                                    t[:, 4 * q:4 * (q + 1), :],
                                    rap[:, 8 * h + 4 * q:
                                        8 * h + 4 * (q + 1), :])
                        else:
                            nc.gpsimd.dma_start(t[:],
                                                rap[:, 8 * h:8 * (h + 1), :])
                        wf[(wname, h)] = t
                for tt in range(N_HW, NT):
                    xb[tt] = stgp.tile([P, E], BF16, name="xb_s",
                                       tag="xbs", bufs=3)
                    nc.gpsimd.dma_start(xb[tt][:],
                                        x_ap[tt * P:(tt + 1) * P, :])

                # --- mask + ones setup -----------------------------------
                nc.vector.memset(maskm[:, 0:512], MASKVAL)
                nc.vector.memset(maskm[:, 512:1024], 0.0)
                nc.gpsimd.affine_select(
                    out=maskm[:, 512:640], in_=maskm[:, 512:640],
                    compare_op=mybir.AluOpType.is_ge, fill=MASKVAL,
                    base=0, pattern=[[1, P]], channel_multiplier=-1,
                )
                for tt in range(NT):
                    nc.vector.memset(
                        v_sb[:, tt, D:D + 1].bitcast(mybir.dt.float32), 1.0)
                    nc.vector.memset(
                        v_sb[:, tt, D + 1:D + 2].bitcast(mybir.dt.float32),
                        0.0)

                # --- x pipeline: load f32 (sync) -> cast bf16 (ACT) ------
                xf = {}

                def load_x(tt):
                    xf[tt] = stgp.tile([P, E], F32, name="x_f", tag="xf",
                                       bufs=2)
                    nc.sync.dma_start(xf[tt][:],
                                      x_ap[tt * P:(tt + 1) * P, :])

                def cast_x(tt):
                    xb[tt] = stgp.tile([P, E], BF16, name="xb_h",
                                       tag="xbh", bufs=2)
                    for h in range(2):
                        nc.scalar.copy(
                            xb[tt][:, 1024 * h:1024 * (h + 1)],
                            xf[tt][:, 1024 * h:1024 * (h + 1)])

                # x0 split in quarters so T0 can start ~2.5us earlier
                xf[0] = stgp.tile([P, E], F32, name="x_f", tag="xf", bufs=2)
                xb[0] = stgp.tile([P, E], BF16, name="xb_h", tag="xbh",
                                  bufs=2)
                for qtr in range(4):
                    nc.sync.dma_start(
                        xf[0][:, 512 * qtr:512 * (qtr + 1)],
                        x_ap[0:P, 512 * qtr:512 * (qtr + 1)])
                load_x(1)
                for qtr in range(4):
                    nc.scalar.copy(
                        xb[0][:, 512 * qtr:512 * (qtr + 1)],
                        xf[0][:, 512 * qtr:512 * (qtr + 1)])
                for tt in range(2, N_HW):
                    load_x(tt)
                    cast_x(tt - 1)
                cast_x(N_HW - 1)

                # --- DVE weight casts, emitted later between PE copies ---
                wb = {"v": wv_b, "k": wk_b, "q": wq_b}

                def cast_w(name, h):
                    for c in range(2):
                        nc.vector.tensor_copy(
                            wb[name][:, 8 * h + 4 * c:8 * h + 4 * (c + 1), :],
                            wf[(name, h)][:, 4 * c:4 * (c + 1), :])

                cast_w("v", 0)
                cast_w("v", 1)

                # --- stage 1 PE units ------------------------------------
                def tr(tt):    # transpose x tile tt into xtb (bf16, PE)
                    for g in range(4):
                        tp = s1ps.tile([P, 4, P], BF16, name="tr_ps",
                                       tag="tr", bufs=3)
                        for j in range(4):
                            ec = 4 * g + j
                            nc.tensor.transpose(
                                tp[:, j],
                                xb[tt][:, ec * P:(ec + 1) * P],
                                ident_b[:])
                        nc.vector.tensor_copy(
                            xtb[:, 4 * g:4 * (g + 1), tt * P:(tt + 1) * P],
                            tp[:])

                pvs = {}

                def proj_v_a(tt):
                    pv = pvs[tt] = s1ps.tile([P, D], F32, name="pv",
                                             tag="pv", bufs=2)
                    for ec in range(8):
                        nc.tensor.matmul(
                            pv[:],
                            xtb[:, ec, tt * P:(tt + 1) * P],
                            wv_b[:, ec, :],
                            start=(ec == 0), stop=False)

                def proj_v_b(tt):
                    pv = pvs.pop(tt)
                    for ec in range(8, EC):
                        nc.tensor.matmul(
                            pv[:],
                            xtb[:, ec, tt * P:(tt + 1) * P],
                            wv_b[:, ec, :],
                            start=False, stop=(ec == EC - 1))
                    nc.vector.tensor_copy(v_sb[:, tt, 0:D], pv[:])

                def proj_v(tt):
                    proj_v_a(tt)
                    proj_v_b(tt)

                pps = {}

                def proj_kq_a(wt, b):   # first ec-half of both dc groups
                    pps[b] = []
                    for dc in range(2):
                        pp = s1ps.tile([P, 512], F32, name="pp", tag="pp",
                                       bufs=2)
                        pps[b].append(pp)
                        for ec in range(8):
                            nc.tensor.matmul(
                                pp[:],
                                wt[:, ec, dc * P:(dc + 1) * P],
                                xtb[:, ec, b * 512:(b + 1) * 512],
                                start=(ec == 0), stop=False)

                def proj_kq_b(wt, dst, b):
                    for dc in range(2):
                        pp = pps[b][dc]
                        for ec in range(8, EC):
                            nc.tensor.matmul(
                                pp[:],
                                wt[:, ec, dc * P:(dc + 1) * P],
                                xtb[:, ec, b * 512:(b + 1) * 512],
                                start=False, stop=(ec == EC - 1))
                        nc.vector.tensor_copy(
                            dst[:, dc, b * 512:(b + 1) * 512], pp[:])
                    del pps[b]

                def proj_kq(wt, dst, b):     # 512-col block b (0..3)
                    proj_kq_a(wt, b)
                    proj_kq_b(wt, dst, b)

                # PE order matched to expected arrival times.
                # x(tt) casts ready ~ 13+3*tt us (HW) / ~42,47,52 (SWDGE);
                # wv ~15/19, wk ~30, wq ~39 (then DVE chunk casts).
                for u in ("T0 V0a T1 V0b V1a T2 V1b V2 T3 V3 CK0 "
                          "T4 V4 K0a CK1 T5 V5 K0b T6 V6 CQ0 Q0a T7 V7 "
                          "CQ1 Q0b T8 V8 T9 V9 K1 T10 V10 T11 V11 Q1 "
                          "T12 V12 T13 V13 K2 T14 V14 Q2 T15 V15 K3 "
                          "Q3").split():
                    if u in ("CK0", "CK1"):
                        cast_w("k", int(u[2]))
                    elif u in ("CQ0", "CQ1"):
                        cast_w("q", int(u[2]))
                    elif u == "K0a":
                        proj_kq_a(wk_b, 0)
                    elif u == "K0b":
                        proj_kq_b(wk_b, kt, 0)
                    elif u == "Q0a":
                        proj_kq_a(wq_b, 0)
                    elif u == "Q0b":
                        proj_kq_b(wq_b, qt, 0)
                    elif u.endswith("a") and u[0] == "V":
                        proj_v_a(int(u[1:-1]))
                    elif u.endswith("b") and u[0] == "V":
                        proj_v_b(int(u[1:-1]))
                    elif u[0] == "T":
                        tr(int(u[1:]))
                    elif u[0] == "V":
                        proj_v(int(u[1:]))
                    elif u[0] == "K":
                        proj_kq(wk_b, kt, int(u[1:]))
                    else:
                        proj_kq(wq_b, qt, int(u[1:]))

            # --- stage 2: causal attention, transposed scores ------------
            # flat stream of (qb, S) steps; scores emitted 3 steps ahead
            steps = [(qb, S) for qb in range(4) for S in range(4 * qb + 4)]

            with tc.tile_pool(name="s2", bufs=1) as s2, \
                 tc.tile_pool(name="s2ps", bufs=1, space="PSUM") as s2ps:
                outs = {}   # (qb, j_t) -> psum tile
                osbs = {}
                scs = {}

                def c0_of(qb, S):
                    j = S - 4 * qb
                    # diag block j fully masks t_loc < 128j: trim to the
                    # widest even >=256 start
                    return 256 if j >= 2 else (128 if j == 1 else 0)

                def scores(qb, S):
                    c0 = c0_of(qb, S)
                    w = 512 - c0
                    sc = s2ps.tile([P, 512], F32, name="sc_ps",
                                   tag="sc", bufs=4)
                    scs[(qb, S)] = sc
                    for dc in range(2):
                        nc.tensor.matmul(
                            sc[:, 0:w],
                            qt[:, dc, S * P:(S + 1) * P],
                            kt[:, dc, qb * QB + c0:(qb + 1) * QB],
                            start=(dc == 0), stop=(dc == 1))
                    j = S - 4 * qb
                    if j >= 0:
                        # mask right after the score matmul, well before exp
                        m0 = c0 + 512 - 128 * j
                        nc.vector.tensor_add(sc[:, 0:w], sc[:, 0:w],
                                             maskm[:, m0:m0 + w])

                def process(qb, S):
                    c0 = c0_of(qb, S)
                    w = 512 - c0
                    j = S - 4 * qb
                    sc = scs.pop((qb, S))
                    p_sb = s2.tile([P, 512], F32R, name="p_sb", tag="p",
                                   bufs=4)
                    nc.scalar.activation(
                        p_sb[:, 0:w], sc[:, 0:w],
                        mybir.ActivationFunctionType.Exp, scale=SCALE)
                    for j_t in range(max(j, 0), 4):
                        if S == 0:
                            outs[(qb, j_t)] = s2ps.tile(
                                [P, D + 2], F32, name="out_ps",
                                tag="out", bufs=4)
                        nc.tensor.matmul(
                            outs[(qb, j_t)][:],
                            p_sb[:, 128 * j_t - c0:128 * j_t - c0 + P],
                            v_sb[:, S, :],
                            start=(S == 0),
                            stop=(S == 4 * qb + j_t))
                    if j >= 0:
                        # t-tile 4*qb+j finished accumulating: normalize
                        op = outs.pop((qb, j))
                        rec = s2.tile([P, 1], F32, name="rec", tag="rec",
                                      bufs=2)
                        nc.vector.reciprocal(rec[:], op[:, D:D + 1])
                        if j == 0:
                            osbs[qb] = s2.tile([P, 4, D], F32, name="o_sb",
                                               tag="osb", bufs=2)
                        nc.vector.tensor_scalar_mul(osbs[qb][:, j, :],
                                                    op[:, 0:D], rec[:])
                        yr = y_ap.rearrange("(tj p) d -> p tj d", p=P)
                        if qb == 3:
                            # last window: per-tile DMAs so only 128KB is
                            # exposed after the final matmul
                            nc.sync.dma_start(yr[:, 4 * qb + j, :],
                                              osbs[qb][:, j, :])
                        elif j == 3:
                            nc.sync.dma_start(yr[:, 4 * qb:4 * (qb + 1), :],
                                              osbs[qb][:])

                for i in range(3):
                    scores(*steps[i])
                for i, st in enumerate(steps):
                    if i + 3 < len(steps):
                        scores(*steps[i + 3])
                    process(*st)

    nc.compile()
    return nc


_NC_CACHE = None


def _get_nc():
    global _NC_CACHE
    if _NC_CACHE is None:
        _NC_CACHE = _build()
    return _NC_CACHE


def run(inputs: dict, trace: bool = False):
    """Run on 8 NeuronCores. Returns (out [8,T,D] f32, exec_time_ns|None)."""
    x = np.ascontiguousarray(np.asarray(inputs["x"], dtype=np.float32))
    wk = np.ascontiguousarray(np.asarray(inputs["Wk"], dtype=np.float32))
    wq = np.ascontiguousarray(np.asarray(inputs["Wq"], dtype=np.float32))
    wv = np.ascontiguousarray(np.asarray(inputs["Wv"], dtype=np.float32))
    assert x.shape == (N_CORES, T, E), x.shape

    nc = _get_nc()
    in_maps = [{"x": x[i], "Wk": wk, "Wq": wq, "Wv": wv}
               for i in range(N_CORES)]
    res = run_bass_kernel_spmd(nc, in_maps, core_ids=list(range(N_CORES)),
                               trace=trace)
    out = np.stack([res.results[i]["out"] for i in range(N_CORES)], axis=0)
    return out, res.exec_time_ns


def kernel(**inputs) -> np.ndarray:
    out, _ = run(inputs, trace=False)
    return out

